# revision 42
# baseline (speedup 1.0000x reference)
"""Trainium2 Bass kernel for the Barrier-Net (DeepSet + control-barrier) model.

Pure data-parallel: x [131072, 68] is sharded along agents across 8
NeuronCores; the small MLP params are folded host-side and replicated.

Per-core math (Na = 16384 agents):
  chunks  = x.reshape(Na, 17, 4)
  u       = sum_c relu(chunk_c @ W1 + b1)                 (phi layer 1)
  r1      = relu(u @ (Wphi2 @ Wrho1) + b')               (phi layer 2 folded into rho layer 1)
  p1      = relu(r1 @ (Wrho2 @ Wpsi1[0:2]) + g @ Wpsi1[2:4] + b'')   (rho layer 2 folded into psi layer 1)
  p2      = relu(p1 @ Wpsi2 + bpsi2)
  empty   = p2 @ Wpsi3 + bpsi3
  barrier = sum_nbr 1/h,  h = sqrt(2*dist - 0.8) + (v.p)/dist, dist = |p_nbr|
  out     = 0.5 * tanh(empty + barrier)

On-chip layout: activations feature-major [feat, agents] for the MLP
(via DMA-xbar transposes of x), agent-major strided APs on the raw x
tile for the barrier term, and psi3 computed agent-major (stationary =
activation tile) so the barrier scalar fuses as a per-partition add.
"""

import os
import sys

import numpy as np

if "/opt/trn_rl_repo" not in sys.path:
    sys.path.insert(0, "/opt/trn_rl_repo")

N_AGENTS = 131072
N_NBR = 16
SD = 4
H = 64
NCORES = 8
NA = N_AGENTS // NCORES          # agents per core
NT = 512                         # agents per MLP tile (matmul moving dim)
GROUP = 1024                     # agents per barrier group
N_TILES = NA // NT               # 32
N_GROUPS = NA // GROUP           # 16
BLOCKS_PER_GROUP = GROUP // 128  # 8
N_BLOCKS = NA // 128             # 128

# number of phi1 pair-psums evacuated on the Scalar engine (rest on Vector)
K_ACT_PAIRS = int(os.environ.get("BARRIER_K_ACT", "4"))
MM_DTYPE = os.environ.get("BARRIER_MM_DTYPE", "f32r")  # f32r | bf16 | f32

_prog_cache = {}


def _round_f32r(a):
    """Round-to-nearest-even at 12 dropped mantissa bits (HW FP32R format)."""
    a = np.ascontiguousarray(np.asarray(a, np.float32))
    bits = a.view(np.uint32)
    drop = 12
    lsb = (bits >> drop) & 1
    rounded = (bits + np.uint32((1 << (drop - 1)) - 1) + lsb) & np.uint32(
        ~((1 << drop) - 1) & 0xFFFFFFFF
    )
    return rounded.view(np.float32)


def _fold_params(params):
    """Host-side weight folding. Returns dict of np.float32 const arrays."""
    f = lambda a: np.asarray(a, np.float32)
    (W1, b1), (W2, b2) = [(f(w), f(b)) for (w, b) in params["phi"]]
    (Wr1, br1), (Wr2, br2) = [(f(w), f(b)) for (w, b) in params["rho"]]
    (Wp1, bp1), (Wp2, bp2), (Wp3, bp3) = [(f(w), f(b)) for (w, b) in params["psi"]]

    n_tot = 1 + N_NBR  # 17

    # phi1 runs on a bf16 x path; full f32 precision is recovered by a
    # hi/lo split of BOTH operands packed along K:
    #   z = W1hi.T x_hi + W1lo.T x_hi + W1hi.T x_lo   (lo*lo term ~2^-16^2)
    # Each 32-row rhs slot holds [x_hi | x_hi | x_lo | x_hi] 8-row groups, so
    # one K=24 matmul per chunk-pair does all three products.
    import ml_dtypes

    W1hi = W1.astype(ml_dtypes.bfloat16).astype(np.float32)
    W1lo = W1 - W1hi
    # pair lhsT [96,128]: at offsets 32q: rows+0:8 blockdiag(W1hi),
    # +8:16 blockdiag(W1lo), +16:24 blockdiag(W1hi)
    W1TRIP = np.zeros((96, 128), np.float32)
    for q in range(3):
        for r, w in ((0, W1hi), (1, W1lo), (2, W1hi)):
            W1TRIP[32 * q + 8 * r : 32 * q + 8 * r + 4, 0:64] = w
            W1TRIP[32 * q + 8 * r + 4 : 32 * q + 8 * r + 8, 64:128] = w
    # odd 17th chunk lhsT: rows 64:88 used (base 64), 4-feat groups with the
    # junk feats of the C-tile zero-masked
    W1C16 = np.zeros((96, 128), np.float32)
    for r, w in ((0, W1hi), (1, W1lo), (2, W1hi)):
        W1C16[64 + 8 * r : 64 + 8 * r + 4, 0:64] = w

    # phi2 folded into rho1: r1 = relu(u @ Wfold + bfold)
    Wfold = W2 @ Wr1                               # [64, 64]
    bfold = (n_tot * b2) @ Wr1 + br1               # [64]
    WfoldS = np.concatenate([Wfold, Wfold], 0)     # [128, 64] (even/odd halves)

    # rho2 folded into psi1: p1 = relu(r1 @ Wa + g @ Wb + bpsi1f)
    # g rides the hi/lo bf16 x path: WBHL [18,64] hits x_hi rows 0:2 with
    # Wb_hi and x_lo rows 16:18 with Wb_lo (ignoring the tiny lo*lo term)
    Wa = Wr2 @ Wp1[0:2, :]                         # [64, 64]
    Wb = np.ascontiguousarray(Wp1[2:4, :])         # [2, 64]
    Wbhi = Wb.astype(ml_dtypes.bfloat16).astype(np.float32)
    WBHL = np.zeros((18, 64), np.float32)
    WBHL[0:2] = Wbhi           # g_hi @ Wb_hi
    WBHL[8:10] = Wb - Wbhi     # g_hi @ Wb_lo  (rows 8:10 = hi-dup slot)
    WBHL[16:18] = Wbhi         # g_lo @ Wb_hi
    bpsi1f = br2 @ Wp1[0:2, :] + bp1               # [64]

    return {
        "W1TRIP": W1TRIP,
        "W1C16": W1C16,
        "WBHL": WBHL,
        "WfoldS": _round_f32r(WfoldS),
        "Wa": _round_f32r(Wa),
        "Wp2": _round_f32r(Wp2),
        "Wp3": np.ascontiguousarray(Wp3),         # [64, 2]
        "b1": b1,
        "bfold": bfold,
        "bpsi1f": bpsi1f,
        "bp2": bp2,
        "bp3": bp3,
    }


def _build_program(consts):
    import concourse.bass as bass
    import concourse.mybir as mybir
    import concourse.tile as tile
    from concourse import bacc
    from concourse.tile_rust import add_dep_helper

    f32 = mybir.dt.float32
    bf16 = mybir.dt.bfloat16
    AF = mybir.ActivationFunctionType
    ALU = mybir.AluOpType

    f32r = mybir.dt.float32r
    has_b1 = bool(np.any(consts["b1"]))
    has_bp3 = bool(np.any(consts["bp3"]))

    nc = bacc.Bacc()
    x_in = nc.declare_dram_parameter("x", [NA, 68], f32, isOutput=False)
    out_d = nc.declare_dram_parameter("out", [NA, 2], f32, isOutput=True)
    cw = {}
    # phi1 weights ride the bf16 x path; rho1/psi1/psi2 run in FP32R
    # (TF32-like, 11-bit mantissa); psi3 stays full fp32 (N=2, cheap)
    cdtypes = {"W1TRIP": bf16, "W1C16": bf16, "WBHL": bf16,
               "WfoldS": f32r, "Wa": f32r, "Wp2": f32r, "Wp3": f32}
    for name, dt_ in cdtypes.items():
        cw[name] = nc.declare_dram_parameter(
            name, list(consts[name].shape), dt_, isOutput=False,
        )
    bias_names = []
    if has_b1:
        bias_names.append("b1pair")  # [128, 1]
    bias_names += ["bfold", "bpsi1f", "bp2"]  # [64, 1] each
    for name in bias_names:
        shp = [128, 1] if name == "b1pair" else [64, 1]
        cw[name] = nc.declare_dram_parameter(name, shp, f32, isOutput=False)

    with tile.TileContext(nc) as tc:
        with (
            tc.tile_pool(name="const", bufs=1) as cpool,
            tc.tile_pool(name="xam", bufs=N_GROUPS, space="SBUF") as xam_pool,
            tc.tile_pool(name="xamh", bufs=3) as xamh_pool,
            tc.tile_pool(name="xt", bufs=4) as xt_pool,
            tc.tile_pool(name="acc", bufs=3) as acc_pool,
            tc.tile_pool(name="apair", bufs=K_ACT_PAIRS + 2 if K_ACT_PAIRS else 1) as apair_pool,
            tc.tile_pool(name="mlp", bufs=3) as mlp_pool,
            tc.tile_pool(name="bsc", bufs=3) as bsc_pool,
            tc.tile_pool(name="bar", bufs=3) as bar_pool,
            tc.tile_pool(name="pre", bufs=1) as pre_pool,
            tc.tile_pool(name="dstage", bufs=N_GROUPS, space="DRAM") as dram_pool,
            tc.tile_pool(name="ppair", bufs=3, space="PSUM") as ppair_pool,
            tc.tile_pool(name="pmlp", bufs=3, space="PSUM") as pmlp_pool,
            tc.tile_pool(name="ppsi3", bufs=2, space="PSUM") as ppsi3_pool,
        ):
            # ---- load constants into SBUF once ----
            # bias vectors are [P, 1] but allocated [P, 4]: a [P, 1] tile's
            # backing tensor loses its free dim and fails AP lowering
            ct = {}
            for name, dram in cw.items():
                if name in bias_names:
                    p = dram.shape[0]
                    t = cpool.tile([p, 4], dram.dtype, tag=name)
                    nc.sync.dma_start(out=t[:, 0:1], in_=dram[:])
                    ct[name] = t[:, 0:1]
                else:
                    t = cpool.tile(list(consts[name].shape), dram.dtype, tag=name)
                    nc.sync.dma_start(out=t[:], in_=dram[:])
                    ct[name] = t

            # persistent pre-tanh buffer [128, 2 * N_BLOCKS]
            pre = pre_pool.tile([128, 2 * N_BLOCKS], f32)

            # Multi-wait DMAs must go through the dynamic-descriptor queue
            # (static DIRECT2D descriptors encode only one semaphore wait);
            # a register-zero DRAM offset forces the dynamic path.
            zero_sv = nc.gpsimd.snap(0)

            # DMA instructions can encode only ONE semaphore wait; NOP
            # "absorbers" on the issuing sequencer carry multi-source waits.
            stage_hist = []    # per group: list of 3 staging DMA insts
            xt_readers_hist = []  # per group: list of matmul insts reading xt
            XT_BUFS = 4

            for g in range(N_GROUPS):
                a0 = g * GROUP
                # ---- agent-major load for the barrier term ----
                # 16 pad cols keep the overlapped bf16 re-layout reads in-tile
                xam = xam_pool.tile([128, 68 * BLOCKS_PER_GROUP + 16], f32)
                src = x_in[a0 : a0 + GROUP, :].rearrange("(b p) f -> p b f", p=128)
                xamf = xam[:, 0 : 68 * BLOCKS_PER_GROUP].rearrange(
                    "p (b f) -> p b f", f=68
                )
                xam_dma = nc.gpsimd.dma_start(out=xamf, in_=src)
                nc.gpsimd.dma_start(
                    out=xam[:, 68 * BLOCKS_PER_GROUP :],
                    in_=x_in[a0 : a0 + 128, 0:16].rearrange("(b p) f -> p b f", p=128),
                )
                # bf16 re-layouts feeding the feature-major transposes: tile X
                # (X=A,B,C; feat offset 0/24/48) holds, per 128-agent block,
                # feats off+8q..off+8q+8 at cols 32q.. (repeated to fill — the
                # transposes read whole 128-col blocks, so no col may be junk)
                xts = []
                stage_insts = []
                npool = nc.gpsimd.nop()
                add_dep_helper(npool.ins, xam_dma.ins, sync=True,
                               reason="absorb DMA-completion dep for staging")
                if g >= 3:
                    for si in stage_hist[g - 3]:
                        add_dep_helper(npool.ins, si.ins, sync=True,
                                       reason="absorb staging WAR")
                for tag, off in (("xamA", 0), ("xamB", 24), ("xamC", 48)):
                    t = xamh_pool.tile([128, 128 * BLOCKS_PER_GROUP], bf16, tag=tag)
                    dstv = t[:].rearrange(
                        "p (b q r i) -> p b q r i", b=BLOCKS_PER_GROUP, q=4, i=8
                    )
                    xap = xam[:]
                    srcv = bass.AP(
                        xap.tensor, xap.offset + off,
                        [list(d) for d in xap.ap[:1]]
                        + [[68, BLOCKS_PER_GROUP], [8, 4], [0, 4], [1, 8]],
                    )
                    nc.vector.tensor_copy(dstv, srcv)
                    # overwrite the r=2 slot group with the bf16 residual
                    # lo = x - hi (mixed-dtype subtract, bf16 out)
                    src8 = bass.AP(
                        xap.tensor, xap.offset + off,
                        [list(d) for d in xap.ap[:1]]
                        + [[68, BLOCKS_PER_GROUP], [8, 4], [1, 8]],
                    )
                    nc.vector.tensor_tensor(
                        out=dstv[:, :, :, 2, :], in0=src8, in1=dstv[:, :, :, 0, :],
                        op=ALU.subtract,
                    )
                    # bounce via DRAM: the xbar transpose reads DRAM sources
                    # (SBUF-sourced transposes overflow the XPOSE wait slots)
                    d = dram_pool.tile([128, 128 * BLOCKS_PER_GROUP], bf16, tag="d" + tag)
                    sdma = nc.gpsimd.dma_start(
                        out=d[:][bass.ds(zero_sv, 128), :], in_=t[:]
                    )
                    add_dep_helper(sdma.ins, npool.ins, sync=False,
                                   reason="order staging after absorber")
                    stage_insts.append(sdma)
                    xts.append((d, "xt" + tag))
                stage_hist.append(stage_insts)
                # sync-engine absorber for the transposes' waits
                nsp = nc.sync.nop()
                for si in stage_insts:
                    add_dep_helper(nsp.ins, si.ins, sync=True,
                                   reason="absorb staging dep for transpose")
                if g >= XT_BUFS:
                    for ri in xt_readers_hist[g - XT_BUFS]:
                        add_dep_helper(nsp.ins, ri.ins, sync=True,
                                       reason="absorb xt WAR for transpose")
                xt_readers = []
                xtiles = []
                for d, tag in xts:
                    # one whole-group transpose: out[p, b, j] = src[j, 128b+p]
                    # -> feature-major [slot, agent] across all 8 blocks
                    xt = xt_pool.tile([128, 128 * BLOCKS_PER_GROUP], bf16, tag=tag)
                    tdma = nc.sync.dma_start_transpose(
                        xt[:].rearrange("p (b j) -> p b j", b=BLOCKS_PER_GROUP), d[:]
                    )
                    add_dep_helper(tdma.ins, nsp.ins, sync=False,
                                   reason="order transpose after absorber")
                    xtiles.append(xt)
                xtA, xtB, xtC = xtiles
                xam4 = xam[:, 0 : 68 * BLOCKS_PER_GROUP].rearrange(
                    "p (b c s) -> p b c s", b=BLOCKS_PER_GROUP, s=4
                )
                # neighbor fields: chunks 1..16 of 17
                P2 = xam4[:, :, 1:, 0:2]      # [128, 8, 16, 2] px,py
                V2 = xam4[:, :, 1:, 2:4]      # [128, 8, 16, 2] vx,vy

                # ---- barrier: dist, h, 1/h ----
                sq = bsc_pool.tile([128, BLOCKS_PER_GROUP, 16, 2], f32, tag="sq")
                nc.vector.tensor_tensor(out=sq[:], in0=P2, in1=P2, op=ALU.mult)
                r2 = bsc_pool.tile([128, BLOCKS_PER_GROUP, 16], f32, tag="r2")
                nc.vector.tensor_tensor(
                    out=r2[:], in0=sq[:, :, :, 0], in1=sq[:, :, :, 1], op=ALU.add
                )
                dist = bsc_pool.tile([128, BLOCKS_PER_GROUP, 16], f32, tag="dist")
                nc.scalar.activation(dist[:], r2[:], AF.Sqrt)
                # u = 2*dist - 0.8
                u_t = bsc_pool.tile([128, BLOCKS_PER_GROUP, 16], f32, tag="u")
                nc.vector.tensor_scalar(
                    out=u_t[:], in0=dist[:], scalar1=2.0, scalar2=-0.8,
                    op0=ALU.mult, op1=ALU.add,
                )
                # inner = r2 * u ;  w = sqrt(inner) = dist * sqrt(2 dist - .8)
                nc.vector.tensor_tensor(out=r2[:], in0=r2[:], in1=u_t[:], op=ALU.mult)
                w_t = bsc_pool.tile([128, BLOCKS_PER_GROUP, 16], f32, tag="w")
                nc.scalar.activation(w_t[:], r2[:], AF.Sqrt)
                # q = vx*px + vy*py
                ab = bsc_pool.tile([128, BLOCKS_PER_GROUP, 16, 2], f32, tag="ab")
                nc.vector.tensor_tensor(out=ab[:], in0=V2, in1=P2, op=ALU.mult)
                qt = bsc_pool.tile([128, BLOCKS_PER_GROUP, 16], f32, tag="q")
                nc.vector.tensor_tensor(
                    out=qt[:], in0=ab[:, :, :, 0], in1=ab[:, :, :, 1], op=ALU.add
                )
                # den = w + q ; contrib = dist / den
                nc.vector.tensor_tensor(out=w_t[:], in0=w_t[:], in1=qt[:], op=ALU.add)
                rec = bsc_pool.tile([128, BLOCKS_PER_GROUP, 16], f32, tag="rec")
                nc.vector.reciprocal_approx_fast(out=rec[:], in_=w_t[:])
                nc.vector.tensor_tensor(out=rec[:], in0=rec[:], in1=dist[:], op=ALU.mult)
                bar = bar_pool.tile([128, BLOCKS_PER_GROUP], f32, tag="bar")
                nc.vector.tensor_reduce(
                    out=bar[:], in_=rec[:], axis=mybir.AxisListType.X, op=ALU.add
                )
                if has_bp3:
                    bar_e = bar_pool.tile([128, BLOCKS_PER_GROUP], f32, tag="bar_e")
                    bar_o = bar_pool.tile([128, BLOCKS_PER_GROUP], f32, tag="bar_o")
                    nc.vector.tensor_scalar(
                        out=bar_e[:], in0=bar[:], scalar1=float(consts["bp3"][0]),
                        scalar2=None, op0=ALU.add,
                    )
                    nc.vector.tensor_scalar(
                        out=bar_o[:], in0=bar[:], scalar1=float(consts["bp3"][1]),
                        scalar2=None, op0=ALU.add,
                    )
                else:
                    bar_e = bar
                    bar_o = bar

                # ---- MLP over the two 512-agent tiles of this group ----
                p2_tiles = []
                for half in range(GROUP // NT):
                    t0 = a0 + half * NT
                    sl = slice(NT * half, NT * half + NT)

                    # phi1: 9 pair matmuls -> relu -> accumulate
                    acc = acc_pool.tile([128, NT], f32r)
                    pair_tiles = []
                    for p in range(9):
                        psum = ppair_pool.tile([128, NT], f32, tag="pair")
                        if p < 8:
                            xtile = (xtA, xtB, xtC)[p // 3]
                            q = 32 * (p % 3)
                            mm = nc.tensor.matmul(
                                psum[:], ct["W1TRIP"][q : q + 24, :],
                                xtile[q : q + 24, sl],
                                start=True, stop=True,
                            )
                        else:
                            mm = nc.tensor.matmul(
                                psum[:], ct["W1C16"][64:88, :],
                                xtC[64:88, sl],
                                start=True, stop=True,
                            )
                        xt_readers.append(mm)
                        if has_b1:
                            # general-bias fallback: ACT relu w/ bias, sep tiles
                            pt = apair_pool.tile([128, NT], f32r, tag="apair")
                            nc.scalar.activation(
                                pt[:], psum[:], AF.Relu, bias=ct["b1pair"]
                            )
                            pair_tiles.append(pt)
                        elif p < K_ACT_PAIRS:
                            pt = apair_pool.tile([128, NT], f32r, tag="apair")
                            nc.scalar.activation(pt[:], psum[:], AF.Relu)
                            pair_tiles.append(pt)
                        elif p == K_ACT_PAIRS:
                            nc.vector.tensor_scalar(
                                out=acc[:], in0=psum[:], scalar1=0.0, scalar2=None,
                                op0=ALU.max,
                            )
                        else:
                            nc.vector.scalar_tensor_tensor(
                                out=acc[:], in0=psum[:], scalar=0.0, in1=acc[:],
                                op0=ALU.max, op1=ALU.add,
                            )

                    # rho1 (phi2 folded): accumulate over acc + pair tiles
                    rhs_list = ([] if has_b1 else [acc]) + pair_tiles
                    pr = pmlp_pool.tile([64, NT], f32, tag="mlppsum")
                    for i, rt in enumerate(rhs_list):
                        nc.tensor.matmul(
                            pr[:], ct["WfoldS"][:], rt[:],
                            start=(i == 0), stop=(i == len(rhs_list) - 1),
                        )
                    r1 = mlp_pool.tile([64, NT], f32r, tag="r1")
                    nc.scalar.activation(r1[:], pr[:], AF.Relu, bias=ct["bfold"])

                    # psi1 (rho2 folded): Wa.T @ r1 + Wb.T @ g
                    pp1 = pmlp_pool.tile([64, NT], f32, tag="mlppsum")
                    nc.tensor.matmul(
                        pp1[:], ct["Wa"][:], r1[:],
                        start=True, stop=False,
                    )
                    xt_readers.append(nc.tensor.matmul(
                        pp1[:], ct["WBHL"][:], xtA[0:18, sl],
                        start=False, stop=True,
                    ))
                    p1 = mlp_pool.tile([64, NT], f32r, tag="p1")
                    nc.scalar.activation(p1[:], pp1[:], AF.Relu, bias=ct["bpsi1f"])

                    # psi2
                    pp2 = pmlp_pool.tile([64, NT], f32, tag="mlppsum")
                    nc.tensor.matmul(
                        pp2[:], ct["Wp2"][:], p1[:],
                        start=True, stop=True,
                    )
                    p2t = mlp_pool.tile([64, NT], f32, tag="p2")
                    nc.scalar.activation(p2t[:], pp2[:], AF.Relu, bias=ct["bp2"])
                    p2_tiles.append(p2t)

                # ---- psi3 agent-major + barrier add ----
                ps3 = ppsi3_pool.tile([128, 2 * BLOCKS_PER_GROUP], f32)
                for b in range(BLOCKS_PER_GROUP):
                    p2t = p2_tiles[b // 4]
                    col = 128 * (b % 4)
                    nc.tensor.matmul(
                        ps3[:, 2 * b : 2 * b + 2],
                        p2t[:, col : col + 128],
                        ct["Wp3"][:],
                        start=True, stop=True,
                    )
                ps3v = ps3[:].rearrange("p (b f) -> p b f", f=2)
                prev = pre[:, 2 * g * BLOCKS_PER_GROUP : 2 * (g + 1) * BLOCKS_PER_GROUP]
                prev = prev.rearrange("p (b f) -> p b f", f=2)
                nc.vector.tensor_tensor(
                    out=prev[:, :, 0], in0=ps3v[:, :, 0], in1=bar_e[:], op=ALU.add
                )
                nc.vector.tensor_tensor(
                    out=prev[:, :, 1], in0=ps3v[:, :, 1], in1=bar_o[:], op=ALU.add
                )
                xt_readers_hist.append(xt_readers)

            # ---- final: tanh, scale, store ----
            fin = pre_pool.tile([128, 2 * N_BLOCKS], f32, tag="fin")
            tanh_i = nc.scalar.activation(fin[:], pre[:], AF.Tanh)
            ts_i = nc.vector.tensor_scalar(
                out=fin[:], in0=fin[:], scalar1=0.5, scalar2=None, op0=ALU.mult
            )
            nfin = nc.gpsimd.nop()
            add_dep_helper(nfin.ins, tanh_i.ins, sync=True, reason="absorb fin deps")
            add_dep_helper(nfin.ins, ts_i.ins, sync=True, reason="absorb fin deps")
            for b in range(N_BLOCKS):
                st = nc.gpsimd.dma_start(
                    out=out_d[128 * b : 128 * (b + 1), :][bass.ds(zero_sv, 128), :],
                    in_=fin[:, 2 * b : 2 * b + 2],
                )
                add_dep_helper(st.ins, nfin.ins, sync=False,
                               reason="order store after absorber")

    nc.finalize()
    return nc


def _get_program(consts):
    key = (K_ACT_PAIRS, MM_DTYPE, bool(np.any(consts["b1"])), bool(np.any(consts["bp3"])))
    if key not in _prog_cache:
        _prog_cache[key] = _build_program(consts)
    return _prog_cache[key]


_patched_walrus = False


def _patch_walrus_flags():
    """Static DMAs in this kernel need >1 semaphore wait (compute-producer +
    DMA-queue deps), which the embedded-descriptor form can't encode. Route
    them through the SP sequencer instead."""
    global _patched_walrus
    if _patched_walrus:
        return
    import concourse.bass_utils as bu

    orig = bu.run_command

    def run_command_patched(argv, **kwargs):
        argv = [
            a.replace("--assign-static-dmas-to-sp=false", "--assign-static-dmas-to-sp=true")
            if isinstance(a, str) else a
            for a in argv
        ]
        return orig(argv, **kwargs)

    bu.run_command = run_command_patched
    _patched_walrus = True


def kernel(x, params):
    from concourse.bass_utils import run_bass_kernel_spmd

    _patch_walrus_flags()

    x = np.ascontiguousarray(np.asarray(x, np.float32))
    assert x.shape == (N_AGENTS, 68), x.shape
    consts = _fold_params(params)
    nc = _get_program(consts)

    import ml_dtypes

    const_map = {}
    for name in ("W1TRIP", "W1C16", "WBHL", "WfoldS", "Wa", "Wp2", "Wp3"):
        a = np.ascontiguousarray(consts[name])
        if name in ("W1TRIP", "W1C16", "WBHL"):
            a = a.astype(ml_dtypes.bfloat16)
        const_map[name] = a
    for name, src in (("bfold", "bfold"), ("bpsi1f", "bpsi1f"), ("bp2", "bp2")):
        const_map[name] = np.ascontiguousarray(consts[src].reshape(64, 1))
    if bool(np.any(consts["b1"])):
        const_map["b1pair"] = np.ascontiguousarray(
            np.concatenate([consts["b1"], consts["b1"]]).reshape(128, 1)
        )

    in_maps = []
    for c in range(NCORES):
        m = dict(const_map)
        m["x"] = np.ascontiguousarray(x[c * NA : (c + 1) * NA])
        in_maps.append(m)

    trace = bool(int(os.environ.get("BARRIER_TRACE", "0")))
    res = run_bass_kernel_spmd(nc, in_maps, list(range(NCORES)), trace=trace)
    if trace and res.exec_time_ns is not None:
        print(f"HW exec time: {res.exec_time_ns} ns")
        if res.mean_exec_time_ns is not None:
            print(f"HW exec time (mean across cores): {res.mean_exec_time_ns:.0f} ns")
    out = np.concatenate([res.results[c]["out"] for c in range(NCORES)], axis=0)
    return out.astype(np.float32)


# revision 43
# speedup vs baseline: 3.1434x; 3.1434x over previous
"""Trainium2 Bass kernel for the Barrier-Net (DeepSet + control-barrier) model.

Pure data-parallel: x [131072, 68] is sharded along agents across 8
NeuronCores; the small MLP params are folded host-side and replicated.

Per-core math (Na = 16384 agents):
  chunks  = x.reshape(Na, 17, 4)
  u       = sum_c relu(chunk_c @ W1 + b1)                 (phi layer 1)
  r1      = relu(u @ (Wphi2 @ Wrho1) + b')               (phi layer 2 folded into rho layer 1)
  p1      = relu(r1 @ (Wrho2 @ Wpsi1[0:2]) + g @ Wpsi1[2:4] + b'')   (rho layer 2 folded into psi layer 1)
  p2      = relu(p1 @ Wpsi2 + bpsi2)
  empty   = p2 @ Wpsi3 + bpsi3
  barrier = sum_nbr 1/h,  h = sqrt(2*dist - 0.8) + (v.p)/dist, dist = |p_nbr|
  out     = 0.5 * tanh(empty + barrier)

On-chip layout: activations feature-major [feat, agents] for the MLP
(via DMA-xbar transposes of x), agent-major strided APs on the raw x
tile for the barrier term, and psi3 computed agent-major (stationary =
activation tile) so the barrier scalar fuses as a per-partition add.
"""

import os
import sys

import numpy as np

if "/opt/trn_rl_repo" not in sys.path:
    sys.path.insert(0, "/opt/trn_rl_repo")

N_AGENTS = 131072
N_NBR = 16
SD = 4
H = 64
NCORES = 8
NA = N_AGENTS // NCORES          # agents per core
NT = 512                         # agents per MLP tile (matmul moving dim)
GROUP = 1024                     # agents per barrier group
N_TILES = NA // NT               # 32
N_GROUPS = NA // GROUP           # 16
BLOCKS_PER_GROUP = GROUP // 128  # 8
N_BLOCKS = NA // 128             # 128

# number of phi1 pair-psums evacuated on the Scalar engine (rest on Vector)
K_ACT_PAIRS = int(os.environ.get("BARRIER_K_ACT", "4"))
MM_DTYPE = os.environ.get("BARRIER_MM_DTYPE", "f32r")  # f32r | bf16 | f32

_prog_cache = {}


def _round_f32r(a):
    """Round-to-nearest-even at 12 dropped mantissa bits (HW FP32R format)."""
    a = np.ascontiguousarray(np.asarray(a, np.float32))
    bits = a.view(np.uint32)
    drop = 12
    lsb = (bits >> drop) & 1
    rounded = (bits + np.uint32((1 << (drop - 1)) - 1) + lsb) & np.uint32(
        ~((1 << drop) - 1) & 0xFFFFFFFF
    )
    return rounded.view(np.float32)


def _fold_params(params):
    """Host-side weight folding. Returns dict of np.float32 const arrays."""
    f = lambda a: np.asarray(a, np.float32)
    (W1, b1), (W2, b2) = [(f(w), f(b)) for (w, b) in params["phi"]]
    (Wr1, br1), (Wr2, br2) = [(f(w), f(b)) for (w, b) in params["rho"]]
    (Wp1, bp1), (Wp2, bp2), (Wp3, bp3) = [(f(w), f(b)) for (w, b) in params["psi"]]

    n_tot = 1 + N_NBR  # 17

    # phi1 runs on a bf16 x path; full f32 precision is recovered by a
    # hi/lo split of BOTH operands packed along K:
    #   z = W1hi.T x_hi + W1lo.T x_hi + W1hi.T x_lo   (lo*lo term ~2^-16^2)
    # Each 32-row rhs slot holds [x_hi | x_hi | x_lo | x_hi] 8-row groups, so
    # one K=24 matmul per chunk-pair does all three products.
    import ml_dtypes

    W1hi = W1.astype(ml_dtypes.bfloat16).astype(np.float32)
    W1lo = W1 - W1hi
    # pair lhsT [96,128]: at offsets 32q: rows+0:8 blockdiag(W1hi),
    # +8:16 blockdiag(W1lo), +16:24 blockdiag(W1hi)
    W1TRIP = np.zeros((96, 128), np.float32)
    for q in range(3):
        for r, w in ((0, W1hi), (1, W1lo), (2, W1hi)):
            W1TRIP[32 * q + 8 * r : 32 * q + 8 * r + 4, 0:64] = w
            W1TRIP[32 * q + 8 * r + 4 : 32 * q + 8 * r + 8, 64:128] = w
    # odd 17th chunk lhsT: rows 64:88 used (base 64), 4-feat groups with the
    # junk feats of the C-tile zero-masked
    W1C16 = np.zeros((96, 128), np.float32)
    for r, w in ((0, W1hi), (1, W1lo), (2, W1hi)):
        W1C16[64 + 8 * r : 64 + 8 * r + 4, 0:64] = w

    # phi2 folded into rho1: r1 = relu(u @ Wfold + bfold)
    Wfold = W2 @ Wr1                               # [64, 64]
    bfold = (n_tot * b2) @ Wr1 + br1               # [64]
    WfoldS = np.concatenate([Wfold, Wfold], 0)     # [128, 64] (even/odd halves)

    # rho2 folded into psi1: p1 = relu(r1 @ Wa + g @ Wb + bpsi1f)
    # g rides the hi/lo bf16 x path: WBHL [18,64] hits x_hi rows 0:2 with
    # Wb_hi and x_lo rows 16:18 with Wb_lo (ignoring the tiny lo*lo term)
    Wa = Wr2 @ Wp1[0:2, :]                         # [64, 64]
    Wb = np.ascontiguousarray(Wp1[2:4, :])         # [2, 64]
    Wbhi = Wb.astype(ml_dtypes.bfloat16).astype(np.float32)
    WBHL = np.zeros((18, 64), np.float32)
    WBHL[0:2] = Wbhi           # g_hi @ Wb_hi
    WBHL[8:10] = Wb - Wbhi     # g_hi @ Wb_lo  (rows 8:10 = hi-dup slot)
    WBHL[16:18] = Wbhi         # g_lo @ Wb_hi
    bpsi1f = br2 @ Wp1[0:2, :] + bp1               # [64]

    return {
        "W1TRIP": W1TRIP,
        "W1C16": W1C16,
        "WBHL": WBHL,
        "WfoldS": _round_f32r(WfoldS),
        "Wa": _round_f32r(Wa),
        "Wp2": _round_f32r(Wp2),
        "Wp3": np.ascontiguousarray(Wp3),         # [64, 2]
        "b1": b1,
        "bfold": bfold,
        "bpsi1f": bpsi1f,
        "bp2": bp2,
        "bp3": bp3,
    }


def _build_program(consts):
    import concourse.bass as bass
    import concourse.mybir as mybir
    import concourse.tile as tile
    from concourse import bacc
    from concourse.tile_rust import add_dep_helper

    f32 = mybir.dt.float32
    bf16 = mybir.dt.bfloat16
    AF = mybir.ActivationFunctionType
    ALU = mybir.AluOpType

    f32r = mybir.dt.float32r
    has_b1 = bool(np.any(consts["b1"]))
    has_bp3 = bool(np.any(consts["bp3"]))

    nc = bacc.Bacc()
    x_in = nc.declare_dram_parameter("x", [NA, 68], f32, isOutput=False)
    out_d = nc.declare_dram_parameter("out", [NA, 2], f32, isOutput=True)
    cw = {}
    # phi1 weights ride the bf16 x path; rho1/psi1/psi2 run in FP32R
    # (TF32-like, 11-bit mantissa); psi3 stays full fp32 (N=2, cheap)
    cdtypes = {"W1TRIP": bf16, "W1C16": bf16, "WBHL": bf16,
               "WfoldS": f32r, "Wa": f32r, "Wp2": f32r, "Wp3": f32}
    for name, dt_ in cdtypes.items():
        cw[name] = nc.declare_dram_parameter(
            name, list(consts[name].shape), dt_, isOutput=False,
        )
    bias_names = []
    if has_b1:
        bias_names.append("b1pair")  # [128, 1]
    bias_names += ["bfold", "bpsi1f", "bp2"]  # [64, 1] each
    for name in bias_names:
        shp = [128, 1] if name == "b1pair" else [64, 1]
        cw[name] = nc.declare_dram_parameter(name, shp, f32, isOutput=False)

    with tile.TileContext(nc) as tc:
        with (
            tc.tile_pool(name="const", bufs=1) as cpool,
            tc.tile_pool(name="xam", bufs=N_GROUPS, space="SBUF") as xam_pool,
            tc.tile_pool(name="xamh", bufs=3) as xamh_pool,
            tc.tile_pool(name="xt", bufs=4) as xt_pool,
            tc.tile_pool(name="acc", bufs=3) as acc_pool,
            tc.tile_pool(name="apair", bufs=K_ACT_PAIRS + 2 if K_ACT_PAIRS else 1) as apair_pool,
            tc.tile_pool(name="mlp", bufs=3) as mlp_pool,
            tc.tile_pool(name="bsc", bufs=3) as bsc_pool,
            tc.tile_pool(name="bar", bufs=3) as bar_pool,
            tc.tile_pool(name="pre", bufs=1) as pre_pool,
            tc.tile_pool(name="ppair", bufs=3, space="PSUM") as ppair_pool,
            tc.tile_pool(name="pmlp", bufs=3, space="PSUM") as pmlp_pool,
            tc.tile_pool(name="ppsi3", bufs=2, space="PSUM") as ppsi3_pool,
        ):
            # ---- load constants into SBUF once ----
            # bias vectors are [P, 1] but allocated [P, 4]: a [P, 1] tile's
            # backing tensor loses its free dim and fails AP lowering
            ct = {}
            for name, dram in cw.items():
                if name in bias_names:
                    p = dram.shape[0]
                    t = cpool.tile([p, 4], dram.dtype, tag=name)
                    nc.sync.dma_start(out=t[:, 0:1], in_=dram[:])
                    ct[name] = t[:, 0:1]
                else:
                    t = cpool.tile(list(consts[name].shape), dram.dtype, tag=name)
                    nc.sync.dma_start(out=t[:], in_=dram[:])
                    ct[name] = t

            # persistent pre-tanh buffer [128, 2 * N_BLOCKS]
            pre = pre_pool.tile([128, 2 * N_BLOCKS], f32)

            for g in range(N_GROUPS):
                a0 = g * GROUP
                # ---- agent-major load for the barrier term ----
                # 16 pad cols keep the overlapped bf16 re-layout reads in-tile
                xam = xam_pool.tile([128, 68 * BLOCKS_PER_GROUP + 16], f32)
                src = x_in[a0 : a0 + GROUP, :].rearrange("(b p) f -> p b f", p=128)
                xamf = xam[:, 0 : 68 * BLOCKS_PER_GROUP].rearrange(
                    "p (b f) -> p b f", f=68
                )
                nc.sync.dma_start(out=xamf, in_=src)
                nc.sync.dma_start(
                    out=xam[:, 68 * BLOCKS_PER_GROUP :],
                    in_=x_in[a0 : a0 + 128, 0:16].rearrange("(b p) f -> p b f", p=128),
                )
                # bf16 re-layouts feeding the feature-major transposes: tile X
                # (X=A,B,C; feat offset 0/24/48) holds, per 128-agent block,
                # feats off+8q..off+8q+8 at cols 32q.. (repeated to fill — the
                # transposes read whole 128-col blocks, so no col may be junk)
                xts = []
                for tag, off in (("xamA", 0), ("xamB", 24), ("xamC", 48)):
                    t = xamh_pool.tile([128, 128 * BLOCKS_PER_GROUP], bf16, tag=tag)
                    dstv = t[:].rearrange(
                        "p (b q r i) -> p b q r i", b=BLOCKS_PER_GROUP, q=4, i=8
                    )
                    xap = xam[:]
                    srcv = bass.AP(
                        xap.tensor, xap.offset + off,
                        [list(d) for d in xap.ap[:1]]
                        + [[68, BLOCKS_PER_GROUP], [8, 4], [0, 4], [1, 8]],
                    )
                    nc.vector.tensor_copy(dstv, srcv)
                    # overwrite the r=2 slot group with the bf16 residual
                    # lo = x - hi (mixed-dtype subtract, bf16 out)
                    src8 = bass.AP(
                        xap.tensor, xap.offset + off,
                        [list(d) for d in xap.ap[:1]]
                        + [[68, BLOCKS_PER_GROUP], [8, 4], [1, 8]],
                    )
                    nc.vector.tensor_tensor(
                        out=dstv[:, :, :, 2, :], in0=src8, in1=dstv[:, :, :, 0, :],
                        op=ALU.subtract,
                    )
                    # one whole-group transpose: out[p, b, j] = src[j, 128b+p]
                    # -> feature-major [slot, agent] across all 8 blocks
                    xt = xt_pool.tile([128, 128 * BLOCKS_PER_GROUP], bf16, tag="xt" + tag)
                    nc.sync.dma_start_transpose(
                        xt[:].rearrange("p (b j) -> p b j", b=BLOCKS_PER_GROUP), t[:]
                    )
                    xts.append(xt)
                xtA, xtB, xtC = xts
                xam4 = xam[:, 0 : 68 * BLOCKS_PER_GROUP].rearrange(
                    "p (b c s) -> p b c s", b=BLOCKS_PER_GROUP, s=4
                )
                # neighbor fields: chunks 1..16 of 17
                P2 = xam4[:, :, 1:, 0:2]      # [128, 8, 16, 2] px,py
                V2 = xam4[:, :, 1:, 2:4]      # [128, 8, 16, 2] vx,vy

                # ---- barrier: dist, h, 1/h ----
                sq = bsc_pool.tile([128, BLOCKS_PER_GROUP, 16, 2], f32, tag="sq")
                nc.vector.tensor_tensor(out=sq[:], in0=P2, in1=P2, op=ALU.mult)
                r2 = bsc_pool.tile([128, BLOCKS_PER_GROUP, 16], f32, tag="r2")
                nc.vector.tensor_tensor(
                    out=r2[:], in0=sq[:, :, :, 0], in1=sq[:, :, :, 1], op=ALU.add
                )
                dist = bsc_pool.tile([128, BLOCKS_PER_GROUP, 16], f32, tag="dist")
                nc.scalar.activation(dist[:], r2[:], AF.Sqrt)
                # u = 2*dist - 0.8
                u_t = bsc_pool.tile([128, BLOCKS_PER_GROUP, 16], f32, tag="u")
                nc.vector.tensor_scalar(
                    out=u_t[:], in0=dist[:], scalar1=2.0, scalar2=-0.8,
                    op0=ALU.mult, op1=ALU.add,
                )
                # inner = r2 * u ;  w = sqrt(inner) = dist * sqrt(2 dist - .8)
                nc.vector.tensor_tensor(out=r2[:], in0=r2[:], in1=u_t[:], op=ALU.mult)
                w_t = bsc_pool.tile([128, BLOCKS_PER_GROUP, 16], f32, tag="w")
                nc.scalar.activation(w_t[:], r2[:], AF.Sqrt)
                # q = vx*px + vy*py
                ab = bsc_pool.tile([128, BLOCKS_PER_GROUP, 16, 2], f32, tag="ab")
                nc.vector.tensor_tensor(out=ab[:], in0=V2, in1=P2, op=ALU.mult)
                qt = bsc_pool.tile([128, BLOCKS_PER_GROUP, 16], f32, tag="q")
                nc.vector.tensor_tensor(
                    out=qt[:], in0=ab[:, :, :, 0], in1=ab[:, :, :, 1], op=ALU.add
                )
                # den = w + q ; contrib = dist / den
                nc.vector.tensor_tensor(out=w_t[:], in0=w_t[:], in1=qt[:], op=ALU.add)
                rec = bsc_pool.tile([128, BLOCKS_PER_GROUP, 16], f32, tag="rec")
                nc.vector.reciprocal_approx_fast(out=rec[:], in_=w_t[:])
                nc.vector.tensor_tensor(out=rec[:], in0=rec[:], in1=dist[:], op=ALU.mult)
                bar = bar_pool.tile([128, BLOCKS_PER_GROUP], f32, tag="bar")
                nc.vector.tensor_reduce(
                    out=bar[:], in_=rec[:], axis=mybir.AxisListType.X, op=ALU.add
                )
                if has_bp3:
                    bar_e = bar_pool.tile([128, BLOCKS_PER_GROUP], f32, tag="bar_e")
                    bar_o = bar_pool.tile([128, BLOCKS_PER_GROUP], f32, tag="bar_o")
                    nc.vector.tensor_scalar(
                        out=bar_e[:], in0=bar[:], scalar1=float(consts["bp3"][0]),
                        scalar2=None, op0=ALU.add,
                    )
                    nc.vector.tensor_scalar(
                        out=bar_o[:], in0=bar[:], scalar1=float(consts["bp3"][1]),
                        scalar2=None, op0=ALU.add,
                    )
                else:
                    bar_e = bar
                    bar_o = bar

                # ---- MLP over the two 512-agent tiles of this group ----
                p2_tiles = []
                for half in range(GROUP // NT):
                    t0 = a0 + half * NT
                    sl = slice(NT * half, NT * half + NT)

                    # phi1: 9 pair matmuls -> relu -> accumulate
                    acc = acc_pool.tile([128, NT], f32r)
                    pair_tiles = []
                    for p in range(9):
                        psum = ppair_pool.tile([128, NT], f32, tag="pair")
                        if p < 8:
                            xtile = (xtA, xtB, xtC)[p // 3]
                            q = 32 * (p % 3)
                            nc.tensor.matmul(
                                psum[:], ct["W1TRIP"][q : q + 24, :],
                                xtile[q : q + 24, sl],
                                start=True, stop=True,
                            )
                        else:
                            nc.tensor.matmul(
                                psum[:], ct["W1C16"][64:88, :],
                                xtC[64:88, sl],
                                start=True, stop=True,
                            )
                        if has_b1:
                            # general-bias fallback: ACT relu w/ bias, sep tiles
                            pt = apair_pool.tile([128, NT], f32r, tag="apair")
                            nc.scalar.activation(
                                pt[:], psum[:], AF.Relu, bias=ct["b1pair"]
                            )
                            pair_tiles.append(pt)
                        elif p < K_ACT_PAIRS:
                            pt = apair_pool.tile([128, NT], f32r, tag="apair")
                            nc.scalar.activation(pt[:], psum[:], AF.Relu)
                            pair_tiles.append(pt)
                        elif p == K_ACT_PAIRS:
                            nc.vector.tensor_scalar(
                                out=acc[:], in0=psum[:], scalar1=0.0, scalar2=None,
                                op0=ALU.max,
                            )
                        else:
                            nc.vector.scalar_tensor_tensor(
                                out=acc[:], in0=psum[:], scalar=0.0, in1=acc[:],
                                op0=ALU.max, op1=ALU.add,
                            )

                    # rho1 (phi2 folded): accumulate over acc + pair tiles
                    rhs_list = ([] if has_b1 else [acc]) + pair_tiles
                    pr = pmlp_pool.tile([64, NT], f32, tag="mlppsum")
                    for i, rt in enumerate(rhs_list):
                        nc.tensor.matmul(
                            pr[:], ct["WfoldS"][:], rt[:],
                            start=(i == 0), stop=(i == len(rhs_list) - 1),
                        )
                    r1 = mlp_pool.tile([64, NT], f32r, tag="r1")
                    nc.scalar.activation(r1[:], pr[:], AF.Relu, bias=ct["bfold"])

                    # psi1 (rho2 folded): Wa.T @ r1 + Wb.T @ g
                    pp1 = pmlp_pool.tile([64, NT], f32, tag="mlppsum")
                    nc.tensor.matmul(
                        pp1[:], ct["Wa"][:], r1[:],
                        start=True, stop=False,
                    )
                    nc.tensor.matmul(
                        pp1[:], ct["WBHL"][:], xtA[0:18, sl],
                        start=False, stop=True,
                    )
                    p1 = mlp_pool.tile([64, NT], f32r, tag="p1")
                    nc.scalar.activation(p1[:], pp1[:], AF.Relu, bias=ct["bpsi1f"])

                    # psi2
                    pp2 = pmlp_pool.tile([64, NT], f32, tag="mlppsum")
                    nc.tensor.matmul(
                        pp2[:], ct["Wp2"][:], p1[:],
                        start=True, stop=True,
                    )
                    p2t = mlp_pool.tile([64, NT], f32, tag="p2")
                    nc.scalar.activation(p2t[:], pp2[:], AF.Relu, bias=ct["bp2"])
                    p2_tiles.append(p2t)

                # ---- psi3 agent-major + barrier add ----
                ps3 = ppsi3_pool.tile([128, 2 * BLOCKS_PER_GROUP], f32)
                for b in range(BLOCKS_PER_GROUP):
                    p2t = p2_tiles[b // 4]
                    col = 128 * (b % 4)
                    nc.tensor.matmul(
                        ps3[:, 2 * b : 2 * b + 2],
                        p2t[:, col : col + 128],
                        ct["Wp3"][:],
                        start=True, stop=True,
                    )
                ps3v = ps3[:].rearrange("p (b f) -> p b f", f=2)
                prev = pre[:, 2 * g * BLOCKS_PER_GROUP : 2 * (g + 1) * BLOCKS_PER_GROUP]
                prev = prev.rearrange("p (b f) -> p b f", f=2)
                nc.vector.tensor_tensor(
                    out=prev[:, :, 0], in0=ps3v[:, :, 0], in1=bar_e[:], op=ALU.add
                )
                nc.vector.tensor_tensor(
                    out=prev[:, :, 1], in0=ps3v[:, :, 1], in1=bar_o[:], op=ALU.add
                )

            # ---- final: tanh, scale, store ----
            fin = pre_pool.tile([128, 2 * N_BLOCKS], f32, tag="fin")
            tanh_i = nc.scalar.activation(fin[:], pre[:], AF.Tanh)
            ts_i = nc.vector.tensor_scalar(
                out=fin[:], in0=fin[:], scalar1=0.5, scalar2=None, op0=ALU.mult
            )
            for b in range(N_BLOCKS):
                nc.sync.dma_start(
                    out=out_d[128 * b : 128 * (b + 1), :],
                    in_=fin[:, 2 * b : 2 * b + 2],
                )

    nc.finalize()
    return nc


def _get_program(consts):
    key = (K_ACT_PAIRS, MM_DTYPE, bool(np.any(consts["b1"])), bool(np.any(consts["bp3"])))
    if key not in _prog_cache:
        _prog_cache[key] = _build_program(consts)
    return _prog_cache[key]


_patched_walrus = False


def _patch_walrus_flags():
    """Static DMAs in this kernel need >1 semaphore wait (compute-producer +
    DMA-queue deps), which the embedded-descriptor form can't encode. Route
    them through the SP sequencer instead."""
    global _patched_walrus
    if _patched_walrus:
        return
    import concourse.bass_utils as bu

    orig = bu.run_command

    def run_command_patched(argv, **kwargs):
        argv = [
            a.replace("--assign-static-dmas-to-sp=false", "--assign-static-dmas-to-sp=true")
            if isinstance(a, str) else a
            for a in argv
        ]
        return orig(argv, **kwargs)

    bu.run_command = run_command_patched
    _patched_walrus = True


def kernel(x, params):
    from concourse.bass_utils import run_bass_kernel_spmd

    _patch_walrus_flags()

    x = np.ascontiguousarray(np.asarray(x, np.float32))
    assert x.shape == (N_AGENTS, 68), x.shape
    consts = _fold_params(params)
    nc = _get_program(consts)

    import ml_dtypes

    const_map = {}
    for name in ("W1TRIP", "W1C16", "WBHL", "WfoldS", "Wa", "Wp2", "Wp3"):
        a = np.ascontiguousarray(consts[name])
        if name in ("W1TRIP", "W1C16", "WBHL"):
            a = a.astype(ml_dtypes.bfloat16)
        const_map[name] = a
    for name, src in (("bfold", "bfold"), ("bpsi1f", "bpsi1f"), ("bp2", "bp2")):
        const_map[name] = np.ascontiguousarray(consts[src].reshape(64, 1))
    if bool(np.any(consts["b1"])):
        const_map["b1pair"] = np.ascontiguousarray(
            np.concatenate([consts["b1"], consts["b1"]]).reshape(128, 1)
        )

    in_maps = []
    for c in range(NCORES):
        m = dict(const_map)
        m["x"] = np.ascontiguousarray(x[c * NA : (c + 1) * NA])
        in_maps.append(m)

    trace = bool(int(os.environ.get("BARRIER_TRACE", "0")))
    res = run_bass_kernel_spmd(nc, in_maps, list(range(NCORES)), trace=trace)
    if trace and res.exec_time_ns is not None:
        print(f"HW exec time: {res.exec_time_ns} ns")
        if res.mean_exec_time_ns is not None:
            print(f"HW exec time (mean across cores): {res.mean_exec_time_ns:.0f} ns")
    out = np.concatenate([res.results[c]["out"] for c in range(NCORES)], axis=0)
    return out.astype(np.float32)
